# revision 21
# baseline (speedup 1.0000x reference)
"""DocRE model kernel for 8 Trainium2 NeuronCores.

Data-parallel over the pair grid: core = b*4 + ib owns document b and
i-rows [8*ib, 8*ib+8) of the 32x32 entity-pair grid (256 pairs/core).
All weights are replicated; W_ext (49152x768, repacked partition-major
on the host) is streamed from HBM in 0.75MB chunks through fp16 matmuls
against group-bilinear tiles built on-chip.  The hs/ts factors are
round-tripped through DRAM so per-group partition-replicated layouts
(hsdup / tsd) can be produced by plain DMAs instead of PE broadcasts.

The four A/B projection blocks are interleaved host-side (m-major) so
each (kc, ct) projection step is one 512-col matmul instead of four
128-col ones.  Tail: LayerNorm is folded into the classifier
  logits = rstd*(relu_feat @ (g.Wc)) - (rstd*mu)*(1'(g.Wc)) + 1'(b.Wc)
and b_ext is added via a rank-1 matmul straight into the PSUM
accumulators.
"""

import numpy as np

import concourse.bacc as bacc
import concourse.bass as bass
import concourse.tile as tile
from concourse import mybir
from concourse.bass_utils import run_bass_kernel_spmd
from concourse.masks import make_identity

F32 = mybir.dt.float32
F16 = mybir.dt.float16

B, L, H = 2, 1024, 768
E, M = 32, 4
EMB, BLK, NL = 768, 64, 97
G = EMB // BLK  # 12
LN_EPS = 1e-12

N_CORES = 8
IB = E // (N_CORES // B)     # 8 i-rows per core
NPAIR = IB * E               # 256 pairs per core
PT = NPAIR // 128            # 2 pair-tiles
KT = EMB * BLK // 128        # 384 k-tiles
WCH = 4                      # k-tiles per W_ext DMA chunk (0.75 MB each)
NCHUNK = KT // WCH           # 96 chunks
WBUFS = 13                   # chunks in flight (~10 MB of SBUF)
CT = EMB // 128              # 6 feature chunks
KC = H // 128                # 6 contraction chunks of H
LP = 384                     # compacted seq rows (<=257 used + padding)
LC = LP // 128               # 3 chunks of LP
NENT = IB + E + 1            # 41 cols: [my 8 entities | all 32 | cls]
NE2 = NENT + 1
CSLAB = 4 * KC * 128         # w4a per-ct slab elements per partition


def _build_module():
    nc = bacc.Bacc("TRN2", target_bir_lowering=False, debug=False)

    seq_d = nc.dram_tensor("seq", [LP, H], F16, kind="ExternalInput")
    S_d = nc.dram_tensor("S", [LP, NENT], F16, kind="ExternalInput")
    # cls-projection blocks (third 768-row block of W_head / W_tail)
    Whc_d = nc.dram_tensor("Whc", [H, EMB], F16, kind="ExternalInput")
    Wtc_d = nc.dram_tensor("Wtc", [H, EMB], F16, kind="ExternalInput")
    # A/B projection blocks, host-packed ct-major: [128, ct, m, kc, 128]
    W4a_d = nc.dram_tensor("W4a", [128, CT * CSLAB], F16, kind="ExternalInput")
    bh_d = nc.dram_tensor("bh", [128, CT], F32, kind="ExternalInput")
    bt_d = nc.dram_tensor("bt", [128, CT], F32, kind="ExternalInput")
    Wx_d = nc.dram_tensor("Wx", [128, KT * EMB], F16, kind="ExternalInput")
    bxr_d = nc.dram_tensor("bxr", [1, EMB], F16, kind="ExternalInput")
    Wgc_d = nc.dram_tensor("Wgc", [EMB, NL], F16, kind="ExternalInput")
    qv_d = nc.dram_tensor("qv", [128, NL], F32, kind="ExternalInput")
    rv_d = nc.dram_tensor("rv", [128, NL], F32, kind="ExternalInput")
    out_d = nc.dram_tensor("out", [NPAIR, NL], F32, kind="ExternalOutput")

    with tile.TileContext(nc) as tc:
        with (
            tc.tile_pool(name="persist", bufs=1) as persist,
            tc.tile_pool(name="seqp", bufs=1) as seqp,
            tc.tile_pool(name="wxp", bufs=WBUFS) as wxp,
            tc.tile_pool(name="blp", bufs=4) as blp,
            tc.tile_pool(name="hsdupp", bufs=2) as hsdupp,
            tc.tile_pool(name="tsdp", bufs=2) as tsdp,
            tc.tile_pool(name="hstp", bufs=3) as hstp,
            tc.tile_pool(name="tmpp", bufs=2) as tmpp,
            tc.tile_pool(name="cnp", bufs=1) as cnp,
            tc.tile_pool(name="dramp", bufs=1, space="DRAM") as dramp,
            tc.tile_pool(name="psf", bufs=1, space="PSUM") as psf,
            tc.tile_pool(name="psg", bufs=3, space="PSUM") as psg,
        ):
            wx_tiles = {}
            wx_next = [0]

            def issue_wx():
                c = wx_next[0]
                t = wxp.tile([128, WCH * EMB], F16, name="wx_ch")
                nc.sync.dma_start(
                    t[:], Wx_d.ap()[:, c * WCH * EMB:(c + 1) * WCH * EMB])
                wx_tiles[c] = t
                wx_next[0] = c + 1

            ident = persist.tile([128, 128], F32, name="ident")
            make_identity(nc, ident[:])

            # ---- head loads.  ALL bulk goes on the sync queue in
            # need-order; the scalar queue is reserved for small
            # latency-critical DMAs (S, factor staging) so they never sit
            # behind megabyte transfers.  Only the ct0 slab of the
            # projection weights is needed before the stream starts.
            seq_t = seqp.tile([128, LC, H], F16, name="seq_t")
            S_t = seqp.tile([128, LC, NENT], F16, name="S_t")
            seq_re = seq_d.ap().rearrange("(c p) h -> p c h", p=128)
            S_re = S_d.ap().rearrange("(c p) n -> p c n", p=128)
            nc.scalar.dma_start(S_t[:], S_re)
            nc.sync.dma_start(seq_t[:], seq_re)

            eps_t = persist.tile([128, 1], F32, name="eps")
            nc.vector.memset(eps_t[:], LN_EPS)

            bh_t = persist.tile([128, CT], F32, name="bh_t")
            bt_t = persist.tile([128, CT], F32, name="bt_t")
            for tile_, src in ((bh_t, bh_d), (bt_t, bt_d)):
                nc.scalar.dma_start(tile_[:], src.ap())

            WC = {}

            def _load_wblock(w_d, name):
                w4 = persist.tile([128, KC, EMB], F16, name=name)
                nc.sync.dma_start(
                    w4[:], w_d.ap().rearrange("(c p) h -> p c h", p=128))
                return w4

            # w4all [128, ct(6), m(4), kc(6), 128], loaded slab-by-slab
            w4all = persist.tile([128, CT, 4, KC, 128], F16, name="w4all")
            w4_flat = w4all[:].rearrange("p c m k h -> p (c m k h)")
            w4_loaded = [False] * CT

            def load_slab(ct):
                nc.sync.dma_start(w4_flat[:, ct * CSLAB:(ct + 1) * CSLAB],
                                  W4a_d.ap()[:, ct * CSLAB:(ct + 1) * CSLAB])
                w4_loaded[ct] = True

            WC[0] = _load_wblock(Whc_d, "wc_hs")
            load_slab(0)
            WC[1] = _load_wblock(Wtc_d, "wc_ts")
            issue_wx()
            load_slab(1)
            issue_wx()
            issue_wx()

            def w4_view(m0, nm, kc, ct):
                # [128, m(nm), 128] slice of w4all at (kc, ct)
                return bass.AP(
                    tensor=w4all.tensor,
                    offset=w4all.offset + ct * CSLAB + m0 * KC * 128 + kc * 128,
                    ap=[w4all.ap[0], [KC * 128, nm], [1, 128]])

            # ---- phase E: entity pooling  ent = S^T @ seq ----
            ps_e0 = psg.tile([NENT, 512], F32, name="gen")
            ps_e1 = psg.tile([NENT, 256], F32, name="gen")
            for kc in range(LC):
                nc.tensor.matmul(ps_e0[:], S_t[:, kc, :], seq_t[:, kc, 0:512],
                                 start=(kc == 0), stop=(kc == LC - 1))
                nc.tensor.matmul(ps_e1[:], S_t[:, kc, :], seq_t[:, kc, 512:768],
                                 start=(kc == 0), stop=(kc == LC - 1))
            ent_nat = persist.tile([NENT, H], F32, name="ent_nat")
            nc.vector.tensor_scalar_mul(ent_nat[:, 0:512], ps_e0[:], 1.0)
            nc.vector.tensor_scalar_mul(ent_nat[:, 512:768], ps_e1[:], 1.0)

            entT = persist.tile([128, KC, NENT], F16, name="entT")
            for kc in range(KC):
                ps_tr = psg.tile([128, NENT], F32, name="gen")
                nc.tensor.transpose(ps_tr[:], ent_nat[:, kc * 128:(kc + 1) * 128],
                                    ident[:NENT, :NENT])
                nc.vector.tensor_scalar_mul(entT[:, kc, :], ps_tr[:], 1.0)

            # ---- phase A: A/B/C projections (batched over m) ----
            ABCD = []
            for ct in range(CT):
                ABCD.append(persist.tile([128, 4, NE2], F32, name=f"abcd{ct}"))

            ps_feat = [[psf.tile([128, 512], F32, name=f"pf{pt}a"),
                        psf.tile([128, 256], F32, name=f"pf{pt}b")]
                       for pt in range(PT)]

            XN = {}

            def emit_proj_mm(ct, half):
                # ps[41, 2, 128] = entT' @ [At|Bt] (half=1) or [Ah|Bh] ct-chunk
                m0 = 2 if half else 0
                ps_n = psg.tile([NENT, 256], F32, name="gen")
                for kc in range(KC):
                    nc.tensor.matmul(ps_n[:], entT[:, kc, :], w4_view(m0, 2, kc, ct),
                                     start=(kc == 0), stop=(kc == KC - 1))
                x_n = tmpp.tile([NENT, 256], F32, name=f"x_n{half}", bufs=2)
                nc.vector.tensor_scalar_mul(x_n[:], ps_n[:], 1.0)
                XN[(ct, half)] = x_n

            def emit_proj_tr(ct, half):
                x_n = XN[(ct, half)]
                for mi in range(2):
                    m = (2 if half else 0) + mi
                    ps_tr = psg.tile([128, NENT], F32, name="gen")
                    nc.tensor.transpose(ps_tr[:], x_n[:, mi * 128:(mi + 1) * 128],
                                        ident[:NENT, :NENT])
                    nc.vector.tensor_scalar_mul(ABCD[ct][:, m, 0:NENT], ps_tr[:], 1.0)

            CB = {}

            def emit_c_chain(side, bias_t):
                # C = cls @ WC[side]; broadcast [1,768] -> [128, CT] via PE
                # transposes (no DRAM round-trip).
                ps_c0 = psg.tile([NENT, 512], F32, name="gen")
                ps_c1 = psg.tile([NENT, 256], F32, name="gen")
                w_t = WC[side]
                for kc in range(KC):
                    nc.tensor.matmul(ps_c0[:1, :], entT[:, kc, IB + E:IB + E + 1],
                                     w_t[:, kc, 0:512],
                                     start=(kc == 0), stop=(kc == KC - 1))
                    nc.tensor.matmul(ps_c1[:1, :], entT[:, kc, IB + E:IB + E + 1],
                                     w_t[:, kc, 512:768],
                                     start=(kc == 0), stop=(kc == KC - 1))
                c_nat = cnp.tile([1, EMB], F32, name="c_nat")
                nc.vector.tensor_scalar_mul(c_nat[:, 0:512], ps_c0[:1, :], 1.0)
                nc.vector.tensor_scalar_mul(c_nat[:, 512:768], ps_c1[:1, :], 1.0)
                cb = persist.tile([128, CT], F32, name=f"cb{side}")
                for ct in range(CT):
                    ps_ctr = psg.tile([128, 1], F32, name="gen")
                    nc.tensor.transpose(ps_ctr[:], c_nat[:, ct * 128:(ct + 1) * 128],
                                        ident[:1, :1])
                    nc.vector.tensor_tensor(cb[:, ct:ct + 1], ps_ctr[:],
                                            bias_t[:, ct:ct + 1],
                                            op=mybir.AluOpType.add)
                CB[side] = cb

            def colview(tile_, m, col0, ap_pat):
                return bass.AP(tensor=tile_.tensor,
                               offset=tile_.offset + m * NE2 + col0,
                               ap=[tile_.ap[0]] + ap_pat)

            ts_dram = dramp.tile([EMB, 256], F16, name="ts_dram")
            hs_dram = dramp.tile([EMB, 256], F16, name="hs_dram")

            def emit_tanh(ct, ma, mb, cola, colb, side, dst_dram, dup_order):
                tmp = tmpp.tile([128, 8, 32], F32, name="tmp")
                nc.vector.tensor_tensor(
                    tmp[:], colview(ABCD[ct], ma, cola[0], cola[1]),
                    colview(ABCD[ct], mb, colb[0], colb[1]),
                    op=mybir.AluOpType.add)
                xt = hstp.tile([128, 256], F16, name="xt")
                nc.scalar.activation(
                    xt[:].rearrange("p (a b) -> p a b", a=8),
                    tmp[:], mybir.ActivationFunctionType.Tanh,
                    bias=CB[side][:, ct:ct + 1], scale=1.0)
                if dup_order:
                    for ph in range(2):
                        dst = bass.AP(
                            tensor=dst_dram.tensor,
                            offset=dst_dram.offset + (ct * 128 + ph * 64) * 256,
                            ap=[[256, 8], [8 * 256, 8], [1, 256]])
                        nc.scalar.dma_start(dst, xt[ph * 64:(ph + 1) * 64, :])
                else:
                    nc.scalar.dma_start(dst_dram[ct * 128:(ct + 1) * 128, :], xt[:])

            # tail constants, emitted mid-stream on the scalar queue.
            wgc_t = persist.tile([128, CT, NL], F16, name="wgc_t")
            qv_b = persist.tile([128, NL], F32, name="qv_b")
            rv_b = persist.tile([128, NL], F32, name="rv_b")
            bx_row = persist.tile([1, EMB], F16, name="bx_row")
            ones_r = persist.tile([1, 128], F16, name="ones_r")
            ident16 = persist.tile([128, 128], F16, name="ident16")

            def emit_tail_consts():
                nc.scalar.dma_start(
                    wgc_t[:], Wgc_d.ap().rearrange("(c p) n -> p c n", p=128))
                nc.scalar.dma_start(qv_b[:], qv_d.ap())
                nc.scalar.dma_start(rv_b[:], rv_d.ap())
                nc.scalar.dma_start(bx_row[:], bxr_d.ap())
                nc.vector.memset(ones_r[:], 1.0)
                nc.scalar.copy(ident16[:], ident[:])

            def emit_tanh_ts(ct):
                emit_tanh(ct, 2, 3, (IB, [[0, 8], [1, 32]]), (0, [[1, 8], [0, 32]]),
                          1, ts_dram, dup_order=False)

            def emit_tanh_hs(ct):
                emit_tanh(ct, 0, 1, (0, [[1, 8], [0, 32]]), (IB, [[0, 8], [1, 32]]),
                          0, hs_dram, dup_order=True)

            # per-ct chain pieces: ts side fully first, hs side after
            def emit_ct_piece(ct, s):
                if s == 0:
                    emit_proj_mm(ct, 1)
                elif s == 1:
                    emit_proj_tr(ct, 1)
                elif s == 2:
                    emit_tanh_ts(ct)
                elif s == 3:
                    emit_proj_mm(ct, 0)
                elif s == 4:
                    emit_proj_tr(ct, 0)
                elif s == 5:
                    emit_tanh_hs(ct)

            # head: hs side first (its 8-DMA staging fan-out gates the
            # first W-matmul), ts side after.
            emit_c_chain(0, bh_t)
            for s in (3, 4, 5):
                emit_ct_piece(0, s)
            emit_c_chain(1, bt_t)
            for s in (0, 1, 2):
                emit_ct_piece(0, s)

            # ---- phase M: main contraction over W_ext ----
            # staging DMAs alternate queues; tsdup replication is 7 flat
            # copies of the seed (depth 1) instead of a serial log-double.
            def emit_hsdup_dma(hsdup, g, di, eng):
                src = bass.AP(
                    tensor=hs_dram.tensor,
                    offset=hs_dram.offset + (g * 64 + di * 8) * 256,
                    ap=[[0, 16], [1, 8 * 256]])
                eng.dma_start(
                    hsdup[di * 16:(di + 1) * 16, :, :].rearrange(
                        "p l c -> p (l c)"), src)

            def emit_tsdup_load(tsdup, g):
                src = bass.AP(
                    tensor=ts_dram.tensor,
                    offset=ts_dram.offset + g * 64 * 256,
                    ap=[[256, 16], [16 * 256, 4], [1, 256]])
                nc.sync.dma_start(tsdup[0:16, :, :], src)

            def emit_tsdup_copy(tsdup, k, eng):
                eng.dma_start(tsdup[16 * k:16 * (k + 1), :, :], tsdup[0:16, :, :])

            def alloc_group():
                return (hsdupp.tile([128, 8, 256], F16, name="hsdup"),
                        tsdp.tile([128, 4, 256], F16, name="tsdup"))

            def stage_group(pair, g, phase):
                # phase 0..3: spread the staging over four slots
                hs, ts = pair
                eng_a, eng_b = nc.scalar, nc.sync
                if phase == 0:
                    emit_tsdup_load(ts, g)
                    emit_hsdup_dma(hs, g, 0, eng_a)
                    emit_hsdup_dma(hs, g, 1, eng_a)
                elif phase == 1:
                    for k in (1, 2, 3):
                        emit_tsdup_copy(ts, k, eng_b if k & 1 else eng_a)
                    emit_hsdup_dma(hs, g, 2, eng_a)
                    emit_hsdup_dma(hs, g, 3, eng_b)
                elif phase == 2:
                    for k in (4, 5):
                        emit_tsdup_copy(ts, k, eng_b if k & 1 else eng_a)
                    emit_hsdup_dma(hs, g, 4, eng_a)
                    emit_hsdup_dma(hs, g, 5, eng_b)
                else:
                    for k in (6, 7):
                        emit_tsdup_copy(ts, k, eng_b if k & 1 else eng_a)
                    emit_hsdup_dma(hs, g, 6, eng_a)
                    emit_hsdup_dma(hs, g, 7, eng_b)

            cur = alloc_group()
            for ph in range(4):
                stage_group(cur, 0, ph)

            for g in range(G):
                nxt = alloc_group() if g + 1 < G else None
                ct_next = g // 2 + 1
                for ib2 in range(8):
                    if nxt is not None and ib2 < 4:
                        stage_group(nxt, g + 1, ib2)
                    if g % 2 == 0 and ct_next < CT and 1 <= ib2 < 7:
                        emit_ct_piece(ct_next, ib2 - 1)
                    # stream the remaining projection slabs 2 groups ahead
                    if (g, ib2) in ((0, 7), (1, 3), (1, 7), (2, 3)):
                        nxt_slab = w4_loaded.index(False) if False in w4_loaded else None
                        if nxt_slab is not None:
                            load_slab(nxt_slab)
                    if g == 1 and ib2 == 6:
                        emit_tail_consts()

                    kt0 = g * 32 + ib2 * 4
                    bl4 = blp.tile([128, 4, 256], F16, name="bl4")
                    hs_b = bass.AP(tensor=cur[0].tensor,
                                   offset=cur[0].offset + ib2 * 256,
                                   ap=[cur[0].ap[0], [0, 4], [1, 256]])
                    nc.vector.tensor_tensor(bl4[:], hs_b, cur[1][:],
                                            op=mybir.AluOpType.mult)
                    # paced refill: at most 2 issues per iteration, ramping
                    # the ring up without a head-of-stream burst.
                    cidx = kt0 // WCH
                    target = min(cidx + WBUFS, NCHUNK)
                    n_iss = min(2, max(0, target - wx_next[0]))
                    for _ in range(n_iss):
                        issue_wx()
                    wx_ch = wx_tiles.pop(cidx)
                    for jb in range(4):
                        kt = kt0 + jb
                        kl = kt % WCH
                        for pt in range(PT):
                            lhsT = bl4[:, jb, pt * 128:(pt + 1) * 128]
                            nc.tensor.matmul(
                                ps_feat[pt][0][:], lhsT,
                                wx_ch[:, kl * EMB:kl * EMB + 512],
                                start=(kt == 0), stop=False)
                            nc.tensor.matmul(
                                ps_feat[pt][1][:], lhsT,
                                wx_ch[:, kl * EMB + 512:(kl + 1) * EMB],
                                start=(kt == 0), stop=False)
                cur = nxt

            # ---- phase L: bias (rank-1 matmul), relu, folded-LN classifier ----
            ln16 = []
            for pt in range(PT):
                nc.tensor.matmul(ps_feat[pt][0][:], ones_r[:], bx_row[:, 0:512],
                                 start=False, stop=True, skip_group_check=True)
                nc.tensor.matmul(ps_feat[pt][1][:], ones_r[:], bx_row[:, 512:768],
                                 start=False, stop=True, skip_group_check=True)
            for pt in range(PT):
                ln = persist.tile([128, EMB], F16, name=f"ln{pt}")
                nc.scalar.activation(ln[:, 0:512], ps_feat[pt][0][:],
                                     mybir.ActivationFunctionType.Relu,
                                     bias=0.0, scale=1.0)
                nc.scalar.activation(ln[:, 512:768], ps_feat[pt][1][:],
                                     mybir.ActivationFunctionType.Relu,
                                     bias=0.0, scale=1.0)
                ln16.append(ln)

            # DVE-only stats first so the ACT queue stays free for the
            # transpose copies; sqrt lands after each pt's copies.
            mvs = []
            for pt in range(PT):
                stats = tmpp.tile([128, 3, 6], F32, name="stats")
                f_re = ln16[pt].rearrange("p (c f) -> p c f", c=3)
                for c in range(3):
                    nc.vector.bn_stats(stats[:, c, :], f_re[:, c, :])
                mv = tmpp.tile([128, 2], F32, name="mv")
                nc.vector.bn_aggr(mv[:], stats[:])
                mvs.append(mv)

            for pt in range(PT):
                lnT = persist.tile([128, CT, 128], F16, name=f"lnT{pt}")
                for ct in range(CT):
                    ps_tr2 = psg.tile([128, 128], F16, name="gen")
                    nc.tensor.transpose(ps_tr2[:], ln16[pt][:, ct * 128:(ct + 1) * 128],
                                        ident16[:])
                    nc.scalar.copy(lnT[:, ct, :], ps_tr2[:])

                ps_lg = psg.tile([128, NL], F32, name="gen")
                for ct in range(CT):
                    nc.tensor.matmul(ps_lg[:], lnT[:, ct, :], wgc_t[:, ct, :],
                                     start=(ct == 0), stop=(ct == CT - 1))
                sd = tmpp.tile([128, 1], F32, name="sd")
                nc.scalar.activation(sd[:], mvs[pt][:, 1:2],
                                     mybir.ActivationFunctionType.Sqrt,
                                     bias=eps_t[:], scale=1.0)
                rstd = tmpp.tile([128, 1], F32, name="rstd")
                nc.vector.reciprocal(rstd[:], sd[:])
                mrs = tmpp.tile([128, 1], F32, name="mrs")
                nc.vector.tensor_tensor(mrs[:], mvs[pt][:, 0:1], rstd[:],
                                        op=mybir.AluOpType.mult)
                # logits = rstd*S1 - mrs*qv + rv
                t1 = tmpp.tile([128, NL], F32, name="t1")
                nc.vector.tensor_scalar(t1[:], ps_lg[:], rstd[:], None,
                                        op0=mybir.AluOpType.mult)
                t2 = tmpp.tile([128, NL], F32, name="t2")
                nc.vector.tensor_scalar(t2[:], qv_b[:], mrs[:], None,
                                        op0=mybir.AluOpType.mult)
                t3 = tmpp.tile([128, NL], F32, name="t3")
                nc.vector.tensor_tensor(t3[:], t1[:], t2[:],
                                        op=mybir.AluOpType.subtract)
                out_sb = tmpp.tile([128, NL], F32, name="out_sb")
                nc.vector.tensor_tensor(out_sb[:], t3[:], rv_b[:],
                                        op=mybir.AluOpType.add)
                nc.scalar.dma_start(out_d.ap()[pt * 128:(pt + 1) * 128, :], out_sb[:])

    nc.compile()
    return nc


_NC_CACHE = []


def _get_module():
    if not _NC_CACHE:
        _NC_CACHE.append(_build_module())
    return _NC_CACHE[0]


def _build_inputs(seq, starts, ends, mention_mask, W_head, b_head, W_tail, b_tail,
                  W_ext, b_ext, ln_g, ln_b, W_cls):
    seq = np.asarray(seq, np.float32)
    starts = np.asarray(starts, np.int64)
    ends = np.asarray(ends, np.int64)
    mask = np.asarray(mention_mask, np.float32)

    S_b = np.zeros((B, L, E), np.float32)
    denom = np.maximum(mask.sum(axis=2), 1.0)          # [B, E]
    w = mask * 0.5 / denom[:, :, None]                 # [B, E, M]
    for b in range(B):
        for e in range(E):
            np.add.at(S_b[b, :, e], starts[b, e] + 1, w[b, e])
            np.add.at(S_b[b, :, e], ends[b, e], w[b, e])

    cls_col = np.zeros((L, 1), np.float32)
    cls_col[0, 0] = 1.0

    ln_g32 = np.asarray(ln_g, np.float32)
    ln_b32 = np.asarray(ln_b, np.float32)
    Wc32 = np.asarray(W_cls, np.float32)
    wgc = (ln_g32[:, None] * Wc32).astype(np.float16)
    qv = (ln_g32[None, :] @ Wc32).astype(np.float32)   # [1, NL]
    rv = (ln_b32[None, :] @ Wc32).astype(np.float32)

    Wh16 = np.asarray(W_head, np.float32).astype(np.float16)
    Wt16 = np.asarray(W_tail, np.float32).astype(np.float16)
    # W4a ct-major: [m, kc, 128p, ct, 128c] -> [128p, ct, m, kc, 128c]
    w4a = np.stack([Wh16[0:H], Wh16[H:2 * H], Wt16[0:H], Wt16[H:2 * H]])
    w4a = (w4a.reshape(4, KC, 128, CT, 128).transpose(2, 3, 0, 1, 4)
           .reshape(128, -1))

    shared = {
        "Whc": np.ascontiguousarray(Wh16[2 * H:3 * H]),
        "Wtc": np.ascontiguousarray(Wt16[2 * H:3 * H]),
        "W4a": np.ascontiguousarray(w4a),
        "bh": np.ascontiguousarray(np.asarray(b_head, np.float32).reshape(CT, 128).T),
        "bt": np.ascontiguousarray(np.asarray(b_tail, np.float32).reshape(CT, 128).T),
        "Wx": np.ascontiguousarray(
            np.asarray(W_ext).astype(np.float16)
            .reshape(G, 8, 8, 4, 16, EMB).transpose(2, 4, 0, 1, 3, 5)
            .reshape(128, KT * EMB)),
        "bxr": np.ascontiguousarray(np.asarray(b_ext, np.float32).astype(np.float16).reshape(1, EMB)),
        "Wgc": np.ascontiguousarray(wgc),
        "qv": np.ascontiguousarray(np.broadcast_to(qv, (128, NL))),
        "rv": np.ascontiguousarray(np.broadcast_to(rv, (128, NL))),
    }
    # compact seq to the rows the pooling matrix actually touches
    rows = [None] * B
    for b in range(B):
        r = np.unique(np.concatenate(
            [starts[b].ravel() + 1, ends[b].ravel(), np.array([0])]))
        assert len(r) <= LP
        rows[b] = r

    in_maps = []
    for core in range(N_CORES):
        b, ib = core // 4, core % 4
        r = rows[b]
        seq_c = np.zeros((LP, H), np.float16)
        seq_c[:len(r)] = seq[b][r].astype(np.float16)
        S_full = np.concatenate(
            [S_b[b][:, ib * IB:(ib + 1) * IB], S_b[b], cls_col], axis=1)
        S_c = np.zeros((LP, NENT), np.float16)
        S_c[:len(r)] = S_full[r].astype(np.float16)
        in_maps.append({
            "seq": np.ascontiguousarray(seq_c),
            "S": np.ascontiguousarray(S_c),
            **shared,
        })
    return in_maps


def kernel(**inputs) -> np.ndarray:
    nc = _get_module()
    in_maps = _build_inputs(**inputs)
    res = run_bass_kernel_spmd(nc, in_maps, core_ids=list(range(N_CORES)))
    outs = np.stack([res.results[c]["out"] for c in range(N_CORES)])  # [8,256,97]
    return outs.reshape(B, 4, IB, E, NL).reshape(B, E, E, NL)


# revision 23
# speedup vs baseline: 1.0982x; 1.0982x over previous
"""DocRE model kernel for 8 Trainium2 NeuronCores.

Data-parallel over the pair grid: core = b*4 + ib owns document b and
i-rows [8*ib, 8*ib+8) of the 32x32 entity-pair grid (256 pairs/core).
All weights are replicated; W_ext (49152x768, repacked partition-major
on the host) is streamed from HBM in 0.75MB chunks through fp16 matmuls
against group-bilinear tiles built on-chip.  The hs/ts factors are
round-tripped through DRAM so per-group partition-replicated layouts
(hsdup / tsd) can be produced by plain DMAs instead of PE broadcasts.

The four A/B projection blocks are interleaved host-side (m-major) so
each (kc, ct) projection step is one 512-col matmul instead of four
128-col ones.  Tail: LayerNorm is folded into the classifier
  logits = rstd*(relu_feat @ (g.Wc)) - (rstd*mu)*(1'(g.Wc)) + 1'(b.Wc)
and b_ext is added via a rank-1 matmul straight into the PSUM
accumulators.
"""

import numpy as np

import concourse.bacc as bacc
import concourse.bass as bass
import concourse.tile as tile
from concourse import mybir
from concourse.bass_utils import run_bass_kernel_spmd
from concourse.masks import make_identity

F32 = mybir.dt.float32
F16 = mybir.dt.float16

B, L, H = 2, 1024, 768
E, M = 32, 4
EMB, BLK, NL = 768, 64, 97
G = EMB // BLK  # 12
LN_EPS = 1e-12

N_CORES = 8
IB = E // (N_CORES // B)     # 8 i-rows per core
NPAIR = IB * E               # 256 pairs per core
PT = NPAIR // 128            # 2 pair-tiles
KT = EMB * BLK // 128        # 384 k-tiles
WCH = 4                      # k-tiles per W_ext DMA chunk (0.75 MB each)
NCHUNK = KT // WCH           # 96 chunks
WBUFS = 13                   # chunks in flight (~10 MB of SBUF)
CT = EMB // 128              # 6 feature chunks
KC = H // 128                # 6 contraction chunks of H
LP = 384                     # compacted seq rows (<=257 used + padding)
LC = LP // 128               # 3 chunks of LP
NENT = IB + E + 1            # 41 cols: [my 8 entities | all 32 | cls]
NE2 = NENT + 1
CSLAB = 4 * KC * 128         # w4a per-ct slab elements per partition


def _build_module():
    nc = bacc.Bacc("TRN2", target_bir_lowering=False, debug=False)

    seq_d = nc.dram_tensor("seq", [LP, H], F16, kind="ExternalInput")
    S_d = nc.dram_tensor("S", [LP, NENT], F16, kind="ExternalInput")
    # cls-projection blocks (third 768-row block of W_head / W_tail)
    Whc_d = nc.dram_tensor("Whc", [H, EMB], F16, kind="ExternalInput")
    Wtc_d = nc.dram_tensor("Wtc", [H, EMB], F16, kind="ExternalInput")
    # A/B projection blocks, host-packed ct-major: [128, ct, m, kc, 128]
    W4a_d = nc.dram_tensor("W4a", [128, CT * CSLAB], F16, kind="ExternalInput")
    bh_d = nc.dram_tensor("bh", [128, CT], F32, kind="ExternalInput")
    bt_d = nc.dram_tensor("bt", [128, CT], F32, kind="ExternalInput")
    Wx_d = nc.dram_tensor("Wx", [128, KT * EMB], F16, kind="ExternalInput")
    bxr_d = nc.dram_tensor("bxr", [1, EMB], F16, kind="ExternalInput")
    Wgc_d = nc.dram_tensor("Wgc", [EMB, NL], F16, kind="ExternalInput")
    qv_d = nc.dram_tensor("qv", [128, NL], F32, kind="ExternalInput")
    rv_d = nc.dram_tensor("rv", [128, NL], F32, kind="ExternalInput")
    out_d = nc.dram_tensor("out", [NPAIR, NL], F32, kind="ExternalOutput")

    with tile.TileContext(nc) as tc:
        with (
            tc.tile_pool(name="persist", bufs=1) as persist,
            tc.tile_pool(name="seqp", bufs=1) as seqp,
            tc.tile_pool(name="wxp", bufs=WBUFS) as wxp,
            tc.tile_pool(name="blp", bufs=4) as blp,
            tc.tile_pool(name="hsdupp", bufs=2) as hsdupp,
            tc.tile_pool(name="tsdp", bufs=2) as tsdp,
            tc.tile_pool(name="hstp", bufs=3) as hstp,
            tc.tile_pool(name="tmpp", bufs=2) as tmpp,
            tc.tile_pool(name="cnp", bufs=1) as cnp,
            tc.tile_pool(name="dramp", bufs=1, space="DRAM") as dramp,
            tc.tile_pool(name="psf", bufs=1, space="PSUM") as psf,
            tc.tile_pool(name="psg", bufs=3, space="PSUM") as psg,
        ):
            wx_tiles = {}
            wx_next = [0]

            def issue_wx():
                c = wx_next[0]
                t = wxp.tile([128, WCH * EMB], F16, name="wx_ch")
                nc.sync.dma_start(
                    t[:], Wx_d.ap()[:, c * WCH * EMB:(c + 1) * WCH * EMB])
                wx_tiles[c] = t
                wx_next[0] = c + 1

            ident = persist.tile([128, 128], F32, name="ident")
            make_identity(nc, ident[:])

            # ---- head loads.  ALL bulk goes on the sync queue in
            # need-order; the scalar queue is reserved for small
            # latency-critical DMAs (S, factor staging) so they never sit
            # behind megabyte transfers.  Only the ct0 slab of the
            # projection weights is needed before the stream starts.
            seq_t = seqp.tile([128, LC, H], F16, name="seq_t")
            S_t = seqp.tile([128, LC, NENT], F16, name="S_t")
            seq_re = seq_d.ap().rearrange("(c p) h -> p c h", p=128)
            S_re = S_d.ap().rearrange("(c p) n -> p c n", p=128)
            nc.scalar.dma_start(S_t[:], S_re)
            nc.sync.dma_start(seq_t[:], seq_re)

            eps_t = persist.tile([128, 1], F32, name="eps")
            nc.vector.memset(eps_t[:], LN_EPS)

            bh_t = persist.tile([128, CT], F32, name="bh_t")
            bt_t = persist.tile([128, CT], F32, name="bt_t")
            for tile_, src in ((bh_t, bh_d), (bt_t, bt_d)):
                nc.scalar.dma_start(tile_[:], src.ap())

            WC = {}

            def _load_wblock(w_d, name):
                w4 = persist.tile([128, KC, EMB], F16, name=name)
                nc.sync.dma_start(
                    w4[:], w_d.ap().rearrange("(c p) h -> p c h", p=128))
                return w4

            # w4all [128, ct(6), m(4), kc(6), 128], loaded slab-by-slab
            w4all = persist.tile([128, CT, 4, KC, 128], F16, name="w4all")
            w4_flat = w4all[:].rearrange("p c m k h -> p (c m k h)")
            w4_loaded = [False] * CT

            def load_slab(ct):
                nc.sync.dma_start(w4_flat[:, ct * CSLAB:(ct + 1) * CSLAB],
                                  W4a_d.ap()[:, ct * CSLAB:(ct + 1) * CSLAB])
                w4_loaded[ct] = True

            WC[0] = _load_wblock(Whc_d, "wc_hs")
            load_slab(0)
            WC[1] = _load_wblock(Wtc_d, "wc_ts")
            issue_wx()
            load_slab(1)
            issue_wx()
            issue_wx()

            def w4_view(m0, nm, kc, ct):
                # [128, m(nm), 128] slice of w4all at (kc, ct)
                return bass.AP(
                    tensor=w4all.tensor,
                    offset=w4all.offset + ct * CSLAB + m0 * KC * 128 + kc * 128,
                    ap=[w4all.ap[0], [KC * 128, nm], [1, 128]])

            # ---- phase E: entity pooling  ent = S^T @ seq ----
            ps_e0 = psg.tile([NENT, 512], F32, name="gen")
            ps_e1 = psg.tile([NENT, 256], F32, name="gen")
            for kc in range(LC):
                nc.tensor.matmul(ps_e0[:], S_t[:, kc, :], seq_t[:, kc, 0:512],
                                 start=(kc == 0), stop=(kc == LC - 1))
                nc.tensor.matmul(ps_e1[:], S_t[:, kc, :], seq_t[:, kc, 512:768],
                                 start=(kc == 0), stop=(kc == LC - 1))
            ent_nat = persist.tile([NENT, H], F32, name="ent_nat")
            nc.vector.tensor_scalar_mul(ent_nat[:, 0:512], ps_e0[:], 1.0)
            nc.vector.tensor_scalar_mul(ent_nat[:, 512:768], ps_e1[:], 1.0)

            entT = persist.tile([128, KC, NENT], F16, name="entT")
            for kc in range(KC):
                ps_tr = psg.tile([128, NENT], F32, name="gen")
                nc.tensor.transpose(ps_tr[:], ent_nat[:, kc * 128:(kc + 1) * 128],
                                    ident[:NENT, :NENT])
                nc.vector.tensor_scalar_mul(entT[:, kc, :], ps_tr[:], 1.0)

            # ---- phase A: A/B/C projections (batched over m) ----
            ABCD = []
            for ct in range(CT):
                ABCD.append(persist.tile([128, 4, NE2], F32, name=f"abcd{ct}"))

            ps_feat = [[psf.tile([128, 512], F32, name=f"pf{pt}a"),
                        psf.tile([128, 256], F32, name=f"pf{pt}b")]
                       for pt in range(PT)]

            XN = {}

            def emit_proj_mm(ct, half):
                # ps[41, 2, 128] = entT' @ [At|Bt] (half=1) or [Ah|Bh] ct-chunk
                m0 = 2 if half else 0
                ps_n = psg.tile([NENT, 256], F32, name="gen")
                for kc in range(KC):
                    nc.tensor.matmul(ps_n[:], entT[:, kc, :], w4_view(m0, 2, kc, ct),
                                     start=(kc == 0), stop=(kc == KC - 1))
                x_n = tmpp.tile([NENT, 256], F32, name=f"x_n{half}", bufs=2)
                nc.vector.tensor_scalar_mul(x_n[:], ps_n[:], 1.0)
                XN[(ct, half)] = x_n

            def emit_proj_tr(ct, half):
                x_n = XN[(ct, half)]
                for mi in range(2):
                    m = (2 if half else 0) + mi
                    ps_tr = psg.tile([128, NENT], F32, name="gen")
                    nc.tensor.transpose(ps_tr[:], x_n[:, mi * 128:(mi + 1) * 128],
                                        ident[:NENT, :NENT])
                    nc.vector.tensor_scalar_mul(ABCD[ct][:, m, 0:NENT], ps_tr[:], 1.0)

            CB = {}

            def emit_c_chain(side, bias_t):
                # C = cls @ WC[side]; broadcast [1,768] -> [128, CT] via PE
                # transposes (no DRAM round-trip).
                ps_c0 = psg.tile([NENT, 512], F32, name="gen")
                ps_c1 = psg.tile([NENT, 256], F32, name="gen")
                w_t = WC[side]
                for kc in range(KC):
                    nc.tensor.matmul(ps_c0[:1, :], entT[:, kc, IB + E:IB + E + 1],
                                     w_t[:, kc, 0:512],
                                     start=(kc == 0), stop=(kc == KC - 1))
                    nc.tensor.matmul(ps_c1[:1, :], entT[:, kc, IB + E:IB + E + 1],
                                     w_t[:, kc, 512:768],
                                     start=(kc == 0), stop=(kc == KC - 1))
                c_nat = cnp.tile([1, EMB], F32, name="c_nat")
                nc.vector.tensor_scalar_mul(c_nat[:, 0:512], ps_c0[:1, :], 1.0)
                nc.vector.tensor_scalar_mul(c_nat[:, 512:768], ps_c1[:1, :], 1.0)
                cb = persist.tile([128, CT], F32, name=f"cb{side}")
                for ct in range(CT):
                    ps_ctr = psg.tile([128, 1], F32, name="gen")
                    nc.tensor.transpose(ps_ctr[:], c_nat[:, ct * 128:(ct + 1) * 128],
                                        ident[:1, :1])
                    nc.vector.tensor_tensor(cb[:, ct:ct + 1], ps_ctr[:],
                                            bias_t[:, ct:ct + 1],
                                            op=mybir.AluOpType.add)
                CB[side] = cb

            def colview(tile_, m, col0, ap_pat):
                return bass.AP(tensor=tile_.tensor,
                               offset=tile_.offset + m * NE2 + col0,
                               ap=[tile_.ap[0]] + ap_pat)

            ts_dram = dramp.tile([EMB, 256], F16, name="ts_dram")
            hs_dram = dramp.tile([EMB, 256], F16, name="hs_dram")

            def emit_tanh(ct, ma, mb, cola, colb, side, dst_dram, dup_order):
                tmp = tmpp.tile([128, 8, 32], F32, name="tmp")
                nc.vector.tensor_tensor(
                    tmp[:], colview(ABCD[ct], ma, cola[0], cola[1]),
                    colview(ABCD[ct], mb, colb[0], colb[1]),
                    op=mybir.AluOpType.add)
                xt = hstp.tile([128, 256], F16, name="xt")
                nc.scalar.activation(
                    xt[:].rearrange("p (a b) -> p a b", a=8),
                    tmp[:], mybir.ActivationFunctionType.Tanh,
                    bias=CB[side][:, ct:ct + 1], scale=1.0)
                if dup_order:
                    for ph in range(2):
                        dst = bass.AP(
                            tensor=dst_dram.tensor,
                            offset=dst_dram.offset + (ct * 128 + ph * 64) * 256,
                            ap=[[256, 8], [8 * 256, 8], [1, 256]])
                        nc.scalar.dma_start(dst, xt[ph * 64:(ph + 1) * 64, :])
                else:
                    nc.scalar.dma_start(dst_dram[ct * 128:(ct + 1) * 128, :], xt[:])

            # tail constants, emitted mid-stream on the scalar queue.
            wgc_t = persist.tile([128, CT, NL], F16, name="wgc_t")
            qv_b = persist.tile([128, NL], F32, name="qv_b")
            rv_b = persist.tile([128, NL], F32, name="rv_b")
            bx_row = persist.tile([1, EMB], F16, name="bx_row")
            ones_r = persist.tile([1, 128], F16, name="ones_r")
            ident16 = persist.tile([128, 128], F16, name="ident16")

            def emit_tail_consts():
                nc.scalar.dma_start(
                    wgc_t[:], Wgc_d.ap().rearrange("(c p) n -> p c n", p=128))
                nc.scalar.dma_start(qv_b[:], qv_d.ap())
                nc.scalar.dma_start(rv_b[:], rv_d.ap())
                nc.scalar.dma_start(bx_row[:], bxr_d.ap())
                nc.vector.memset(ones_r[:], 1.0)
                nc.scalar.copy(ident16[:], ident[:])

            def emit_tanh_ts(ct):
                emit_tanh(ct, 2, 3, (IB, [[0, 8], [1, 32]]), (0, [[1, 8], [0, 32]]),
                          1, ts_dram, dup_order=False)

            def emit_tanh_hs(ct):
                emit_tanh(ct, 0, 1, (0, [[1, 8], [0, 32]]), (IB, [[0, 8], [1, 32]]),
                          0, hs_dram, dup_order=True)

            # per-ct chain pieces: ts side fully first, hs side after
            def emit_ct_piece(ct, s):
                if s == 0:
                    emit_proj_mm(ct, 1)
                elif s == 1:
                    emit_proj_tr(ct, 1)
                elif s == 2:
                    emit_tanh_ts(ct)
                elif s == 3:
                    emit_proj_mm(ct, 0)
                elif s == 4:
                    emit_proj_tr(ct, 0)
                elif s == 5:
                    emit_tanh_hs(ct)

            # head: hs side first (its 8-DMA staging fan-out gates the
            # first W-matmul), ts side after.  ct1 is emitted in the head
            # too so group-2/3 staging never waits on a mid-stream chain.
            emit_c_chain(0, bh_t)
            for s in (3, 4, 5):
                emit_ct_piece(0, s)
            emit_c_chain(1, bt_t)
            for s in (0, 1, 2):
                emit_ct_piece(0, s)
            for s in (0, 3, 1, 4, 2, 5):
                emit_ct_piece(1, s)
            load_slab(2)

            # ---- phase M: main contraction over W_ext ----
            # staging DMAs alternate queues; tsdup replication is 7 flat
            # copies of the seed (depth 1) instead of a serial log-double.
            def emit_hsdup_dma(hsdup, g, di, eng):
                src = bass.AP(
                    tensor=hs_dram.tensor,
                    offset=hs_dram.offset + (g * 64 + di * 8) * 256,
                    ap=[[0, 16], [1, 8 * 256]])
                eng.dma_start(
                    hsdup[di * 16:(di + 1) * 16, :, :].rearrange(
                        "p l c -> p (l c)"), src)

            def emit_tsdup_load(tsdup, g):
                src = bass.AP(
                    tensor=ts_dram.tensor,
                    offset=ts_dram.offset + g * 64 * 256,
                    ap=[[256, 16], [16 * 256, 4], [1, 256]])
                nc.sync.dma_start(tsdup[0:16, :, :], src)

            def emit_tsdup_copy(tsdup, k, eng):
                eng.dma_start(tsdup[16 * k:16 * (k + 1), :, :], tsdup[0:16, :, :])

            def alloc_group():
                return (hsdupp.tile([128, 8, 256], F16, name="hsdup"),
                        tsdp.tile([128, 4, 256], F16, name="tsdup"))

            def stage_group(pair, g, phase):
                # phase 0..3: spread the staging over four slots
                hs, ts = pair
                eng_a, eng_b = nc.scalar, nc.sync
                if phase == 0:
                    emit_tsdup_load(ts, g)
                    emit_hsdup_dma(hs, g, 0, eng_a)
                    emit_hsdup_dma(hs, g, 1, eng_a)
                elif phase == 1:
                    for k in (1, 2, 3):
                        emit_tsdup_copy(ts, k, eng_b if k & 1 else eng_a)
                    emit_hsdup_dma(hs, g, 2, eng_a)
                    emit_hsdup_dma(hs, g, 3, eng_b)
                elif phase == 2:
                    for k in (4, 5):
                        emit_tsdup_copy(ts, k, eng_b if k & 1 else eng_a)
                    emit_hsdup_dma(hs, g, 4, eng_a)
                    emit_hsdup_dma(hs, g, 5, eng_b)
                else:
                    for k in (6, 7):
                        emit_tsdup_copy(ts, k, eng_b if k & 1 else eng_a)
                    emit_hsdup_dma(hs, g, 6, eng_a)
                    emit_hsdup_dma(hs, g, 7, eng_b)

            cur = alloc_group()
            for ph in range(4):
                stage_group(cur, 0, ph)

            for g in range(G):
                nxt = alloc_group() if g + 1 < G else None
                ct_next = g // 2 + 2
                for ib2 in range(8):
                    if nxt is not None and ib2 < 4:
                        stage_group(nxt, g + 1, ib2)
                    if g % 2 == 0 and ct_next < CT and 1 <= ib2 < 7:
                        emit_ct_piece(ct_next, ib2 - 1)
                    # stream the remaining projection slabs ahead of use
                    if (g, ib2) in ((0, 3), (0, 7), (1, 3)):
                        nxt_slab = w4_loaded.index(False) if False in w4_loaded else None
                        if nxt_slab is not None:
                            load_slab(nxt_slab)
                    if g == 1 and ib2 == 6:
                        emit_tail_consts()

                    kt0 = g * 32 + ib2 * 4
                    bl4 = blp.tile([128, 4, 256], F16, name="bl4")
                    hs_b = bass.AP(tensor=cur[0].tensor,
                                   offset=cur[0].offset + ib2 * 256,
                                   ap=[cur[0].ap[0], [0, 4], [1, 256]])
                    nc.vector.tensor_tensor(bl4[:], hs_b, cur[1][:],
                                            op=mybir.AluOpType.mult)
                    # paced refill: at most 2 issues per iteration, ramping
                    # the ring up without a head-of-stream burst.
                    cidx = kt0 // WCH
                    target = min(cidx + WBUFS, NCHUNK)
                    n_iss = min(2, max(0, target - wx_next[0]))
                    for _ in range(n_iss):
                        issue_wx()
                    wx_ch = wx_tiles.pop(cidx)
                    for jb in range(4):
                        kt = kt0 + jb
                        kl = kt % WCH
                        for pt in range(PT):
                            lhsT = bl4[:, jb, pt * 128:(pt + 1) * 128]
                            nc.tensor.matmul(
                                ps_feat[pt][0][:], lhsT,
                                wx_ch[:, kl * EMB:kl * EMB + 512],
                                start=(kt == 0), stop=False)
                            nc.tensor.matmul(
                                ps_feat[pt][1][:], lhsT,
                                wx_ch[:, kl * EMB + 512:(kl + 1) * EMB],
                                start=(kt == 0), stop=False)
                cur = nxt

            # ---- phase L: bias (rank-1 matmul), relu, folded-LN classifier ----
            ln16 = []
            for pt in range(PT):
                nc.tensor.matmul(ps_feat[pt][0][:], ones_r[:], bx_row[:, 0:512],
                                 start=False, stop=True, skip_group_check=True)
                nc.tensor.matmul(ps_feat[pt][1][:], ones_r[:], bx_row[:, 512:768],
                                 start=False, stop=True, skip_group_check=True)
            for pt in range(PT):
                ln = persist.tile([128, EMB], F16, name=f"ln{pt}")
                nc.scalar.activation(ln[:, 0:512], ps_feat[pt][0][:],
                                     mybir.ActivationFunctionType.Relu,
                                     bias=0.0, scale=1.0)
                nc.scalar.activation(ln[:, 512:768], ps_feat[pt][1][:],
                                     mybir.ActivationFunctionType.Relu,
                                     bias=0.0, scale=1.0)
                ln16.append(ln)

            # DVE-only stats first so the ACT queue stays free for the
            # transpose copies; sqrt lands after each pt's copies.
            mvs = []
            for pt in range(PT):
                stats = tmpp.tile([128, 3, 6], F32, name="stats")
                f_re = ln16[pt].rearrange("p (c f) -> p c f", c=3)
                for c in range(3):
                    nc.vector.bn_stats(stats[:, c, :], f_re[:, c, :])
                mv = tmpp.tile([128, 2], F32, name="mv")
                nc.vector.bn_aggr(mv[:], stats[:])
                mvs.append(mv)

            for pt in range(PT):
                lnT = persist.tile([128, CT, 128], F16, name=f"lnT{pt}")
                for ct in range(CT):
                    ps_tr2 = psg.tile([128, 128], F16, name="gen")
                    nc.tensor.transpose(ps_tr2[:], ln16[pt][:, ct * 128:(ct + 1) * 128],
                                        ident16[:])
                    nc.scalar.copy(lnT[:, ct, :], ps_tr2[:])

                ps_lg = psg.tile([128, NL], F32, name="gen")
                for ct in range(CT):
                    nc.tensor.matmul(ps_lg[:], lnT[:, ct, :], wgc_t[:, ct, :],
                                     start=(ct == 0), stop=(ct == CT - 1))
                sd = tmpp.tile([128, 1], F32, name="sd")
                nc.scalar.activation(sd[:], mvs[pt][:, 1:2],
                                     mybir.ActivationFunctionType.Sqrt,
                                     bias=eps_t[:], scale=1.0)
                rstd = tmpp.tile([128, 1], F32, name="rstd")
                nc.vector.reciprocal(rstd[:], sd[:])
                mrs = tmpp.tile([128, 1], F32, name="mrs")
                nc.vector.tensor_tensor(mrs[:], mvs[pt][:, 0:1], rstd[:],
                                        op=mybir.AluOpType.mult)
                # logits = rstd*S1 - mrs*qv + rv
                t1 = tmpp.tile([128, NL], F32, name="t1")
                nc.vector.tensor_scalar(t1[:], ps_lg[:], rstd[:], None,
                                        op0=mybir.AluOpType.mult)
                t2 = tmpp.tile([128, NL], F32, name="t2")
                nc.vector.tensor_scalar(t2[:], qv_b[:], mrs[:], None,
                                        op0=mybir.AluOpType.mult)
                t3 = tmpp.tile([128, NL], F32, name="t3")
                nc.vector.tensor_tensor(t3[:], t1[:], t2[:],
                                        op=mybir.AluOpType.subtract)
                out_sb = tmpp.tile([128, NL], F32, name="out_sb")
                nc.vector.tensor_tensor(out_sb[:], t3[:], rv_b[:],
                                        op=mybir.AluOpType.add)
                nc.scalar.dma_start(out_d.ap()[pt * 128:(pt + 1) * 128, :], out_sb[:])

    nc.compile()
    return nc


_NC_CACHE = []


def _get_module():
    if not _NC_CACHE:
        _NC_CACHE.append(_build_module())
    return _NC_CACHE[0]


def _build_inputs(seq, starts, ends, mention_mask, W_head, b_head, W_tail, b_tail,
                  W_ext, b_ext, ln_g, ln_b, W_cls):
    seq = np.asarray(seq, np.float32)
    starts = np.asarray(starts, np.int64)
    ends = np.asarray(ends, np.int64)
    mask = np.asarray(mention_mask, np.float32)

    S_b = np.zeros((B, L, E), np.float32)
    denom = np.maximum(mask.sum(axis=2), 1.0)          # [B, E]
    w = mask * 0.5 / denom[:, :, None]                 # [B, E, M]
    for b in range(B):
        for e in range(E):
            np.add.at(S_b[b, :, e], starts[b, e] + 1, w[b, e])
            np.add.at(S_b[b, :, e], ends[b, e], w[b, e])

    cls_col = np.zeros((L, 1), np.float32)
    cls_col[0, 0] = 1.0

    ln_g32 = np.asarray(ln_g, np.float32)
    ln_b32 = np.asarray(ln_b, np.float32)
    Wc32 = np.asarray(W_cls, np.float32)
    wgc = (ln_g32[:, None] * Wc32).astype(np.float16)
    qv = (ln_g32[None, :] @ Wc32).astype(np.float32)   # [1, NL]
    rv = (ln_b32[None, :] @ Wc32).astype(np.float32)

    Wh16 = np.asarray(W_head, np.float32).astype(np.float16)
    Wt16 = np.asarray(W_tail, np.float32).astype(np.float16)
    # W4a ct-major: [m, kc, 128p, ct, 128c] -> [128p, ct, m, kc, 128c]
    w4a = np.stack([Wh16[0:H], Wh16[H:2 * H], Wt16[0:H], Wt16[H:2 * H]])
    w4a = (w4a.reshape(4, KC, 128, CT, 128).transpose(2, 3, 0, 1, 4)
           .reshape(128, -1))

    shared = {
        "Whc": np.ascontiguousarray(Wh16[2 * H:3 * H]),
        "Wtc": np.ascontiguousarray(Wt16[2 * H:3 * H]),
        "W4a": np.ascontiguousarray(w4a),
        "bh": np.ascontiguousarray(np.asarray(b_head, np.float32).reshape(CT, 128).T),
        "bt": np.ascontiguousarray(np.asarray(b_tail, np.float32).reshape(CT, 128).T),
        "Wx": np.ascontiguousarray(
            np.asarray(W_ext).astype(np.float16)
            .reshape(G, 8, 8, 4, 16, EMB).transpose(2, 4, 0, 1, 3, 5)
            .reshape(128, KT * EMB)),
        "bxr": np.ascontiguousarray(np.asarray(b_ext, np.float32).astype(np.float16).reshape(1, EMB)),
        "Wgc": np.ascontiguousarray(wgc),
        "qv": np.ascontiguousarray(np.broadcast_to(qv, (128, NL))),
        "rv": np.ascontiguousarray(np.broadcast_to(rv, (128, NL))),
    }
    # compact seq to the rows the pooling matrix actually touches
    rows = [None] * B
    for b in range(B):
        r = np.unique(np.concatenate(
            [starts[b].ravel() + 1, ends[b].ravel(), np.array([0])]))
        assert len(r) <= LP
        rows[b] = r

    in_maps = []
    for core in range(N_CORES):
        b, ib = core // 4, core % 4
        r = rows[b]
        seq_c = np.zeros((LP, H), np.float16)
        seq_c[:len(r)] = seq[b][r].astype(np.float16)
        S_full = np.concatenate(
            [S_b[b][:, ib * IB:(ib + 1) * IB], S_b[b], cls_col], axis=1)
        S_c = np.zeros((LP, NENT), np.float16)
        S_c[:len(r)] = S_full[r].astype(np.float16)
        in_maps.append({
            "seq": np.ascontiguousarray(seq_c),
            "S": np.ascontiguousarray(S_c),
            **shared,
        })
    return in_maps


def kernel(**inputs) -> np.ndarray:
    nc = _get_module()
    in_maps = _build_inputs(**inputs)
    res = run_bass_kernel_spmd(nc, in_maps, core_ids=list(range(N_CORES)))
    outs = np.stack([res.results[c]["out"] for c in range(N_CORES)])  # [8,256,97]
    return outs.reshape(B, 4, IB, E, NL).reshape(B, E, E, NL)


# revision 27
# speedup vs baseline: 1.1357x; 1.0342x over previous
"""DocRE model kernel for 8 Trainium2 NeuronCores.

Data-parallel over the pair grid: core = b*4 + ib owns document b and
i-rows [8*ib, 8*ib+8) of the 32x32 entity-pair grid (256 pairs/core).
All weights are replicated; W_ext (49152x768, repacked partition-major
on the host) is streamed from HBM in 0.75MB chunks through fp16 matmuls
against group-bilinear tiles built on-chip.  The hs/ts factors are
round-tripped through DRAM so per-group partition-replicated layouts
(hsdup / tsd) can be produced by plain DMAs instead of PE broadcasts.

The four A/B projection blocks are interleaved host-side (m-major) so
each (kc, ct) projection step is one 512-col matmul instead of four
128-col ones.  Tail: LayerNorm is folded into the classifier
  logits = rstd*(relu_feat @ (g.Wc)) - (rstd*mu)*(1'(g.Wc)) + 1'(b.Wc)
and b_ext is added via a rank-1 matmul straight into the PSUM
accumulators.
"""

import numpy as np

import concourse.bacc as bacc
import concourse.bass as bass
import concourse.tile as tile
from concourse import mybir
from concourse.bass_utils import run_bass_kernel_spmd
from concourse.masks import make_identity

F32 = mybir.dt.float32
F16 = mybir.dt.float16

B, L, H = 2, 1024, 768
E, M = 32, 4
EMB, BLK, NL = 768, 64, 97
G = EMB // BLK  # 12
LN_EPS = 1e-12

N_CORES = 8
IB = E // (N_CORES // B)     # 8 i-rows per core
NPAIR = IB * E               # 256 pairs per core
PT = NPAIR // 128            # 2 pair-tiles
KT = EMB * BLK // 128        # 384 k-tiles
WCH = 4                      # k-tiles per W_ext DMA chunk (0.75 MB each)
NCHUNK = KT // WCH           # 96 chunks
WBUFS = 13                   # chunks in flight (~10 MB of SBUF)
CT = EMB // 128              # 6 feature chunks
KC = H // 128                # 6 contraction chunks of H
LP = 384                     # compacted seq rows (<=257 used + padding)
LC = LP // 128               # 3 chunks of LP
NENT = IB + E + 1            # 41 cols: [my 8 entities | all 32 | cls]
NE2 = NENT + 1
CSLAB = 4 * KC * 128         # w4a per-ct slab elements per partition


def _build_module():
    nc = bacc.Bacc("TRN2", target_bir_lowering=False, debug=False)

    seq_d = nc.dram_tensor("seq", [LP, H], F16, kind="ExternalInput")
    S_d = nc.dram_tensor("S", [LP, NENT], F16, kind="ExternalInput")
    # cls-projection blocks (third 768-row block of W_head / W_tail)
    Whc_d = nc.dram_tensor("Whc", [H, EMB], F16, kind="ExternalInput")
    Wtc_d = nc.dram_tensor("Wtc", [H, EMB], F16, kind="ExternalInput")
    # A/B projection blocks, host-packed ct-major: [128, ct, m, kc, 128]
    W4a_d = nc.dram_tensor("W4a", [128, CT * CSLAB], F16, kind="ExternalInput")
    bh_d = nc.dram_tensor("bh", [128, CT], F32, kind="ExternalInput")
    bt_d = nc.dram_tensor("bt", [128, CT], F32, kind="ExternalInput")
    Wx_d = nc.dram_tensor("Wx", [128, KT * EMB], F16, kind="ExternalInput")
    bxr_d = nc.dram_tensor("bxr", [1, EMB], F16, kind="ExternalInput")
    Wgc_d = nc.dram_tensor("Wgc", [EMB, NL], F16, kind="ExternalInput")
    qv_d = nc.dram_tensor("qv", [128, NL], F32, kind="ExternalInput")
    rv_d = nc.dram_tensor("rv", [128, NL], F32, kind="ExternalInput")
    out_d = nc.dram_tensor("out", [NPAIR, NL], F32, kind="ExternalOutput")

    with tile.TileContext(nc) as tc:
        with (
            tc.tile_pool(name="persist", bufs=1) as persist,
            tc.tile_pool(name="seqp", bufs=1) as seqp,
            tc.tile_pool(name="wxp", bufs=WBUFS) as wxp,
            tc.tile_pool(name="blp", bufs=4) as blp,
            tc.tile_pool(name="hsdupp", bufs=2) as hsdupp,
            tc.tile_pool(name="tsdp", bufs=2) as tsdp,
            tc.tile_pool(name="hstp", bufs=3) as hstp,
            tc.tile_pool(name="tmpp", bufs=2) as tmpp,
            tc.tile_pool(name="cnp", bufs=1) as cnp,
            tc.tile_pool(name="dramp", bufs=1, space="DRAM") as dramp,
            tc.tile_pool(name="psf", bufs=1, space="PSUM") as psf,
            tc.tile_pool(name="psg", bufs=3, space="PSUM") as psg,
        ):
            wx_tiles = {}
            wx_next = [0]

            def issue_wx():
                c = wx_next[0]
                t = wxp.tile([128, WCH * EMB], F16, name="wx_ch")
                nc.sync.dma_start(
                    t[:], Wx_d.ap()[:, c * WCH * EMB:(c + 1) * WCH * EMB])
                wx_tiles[c] = t
                wx_next[0] = c + 1

            ident = persist.tile([128, 128], F32, name="ident")
            make_identity(nc, ident[:])

            # ---- head loads.  ALL bulk goes on the sync queue in
            # need-order; the scalar queue is reserved for small
            # latency-critical DMAs (S, factor staging) so they never sit
            # behind megabyte transfers.  Only the ct0 slab of the
            # projection weights is needed before the stream starts.
            seq_t = seqp.tile([128, LC, H], F16, name="seq_t")
            S_t = seqp.tile([128, LC, NENT], F16, name="S_t")
            seq_re = seq_d.ap().rearrange("(c p) h -> p c h", p=128)
            S_re = S_d.ap().rearrange("(c p) n -> p c n", p=128)
            nc.scalar.dma_start(S_t[:], S_re)
            nc.sync.dma_start(seq_t[:], seq_re)

            eps_t = persist.tile([128, 1], F32, name="eps")
            nc.vector.memset(eps_t[:], LN_EPS)

            bh_t = persist.tile([128, CT], F32, name="bh_t")
            bt_t = persist.tile([128, CT], F32, name="bt_t")
            for tile_, src in ((bh_t, bh_d), (bt_t, bt_d)):
                nc.scalar.dma_start(tile_[:], src.ap())

            WC = {}

            def _load_wblock(w_d, name):
                w4 = persist.tile([128, KC, EMB], F16, name=name)
                nc.sync.dma_start(
                    w4[:], w_d.ap().rearrange("(c p) h -> p c h", p=128))
                return w4

            # w4all [128, ct(6), m(4), kc(6), 128], loaded slab-by-slab
            w4all = persist.tile([128, CT, 4, KC, 128], F16, name="w4all")
            w4_flat = w4all[:].rearrange("p c m k h -> p (c m k h)")
            w4_loaded = [False] * CT

            def load_slab(ct):
                nc.sync.dma_start(w4_flat[:, ct * CSLAB:(ct + 1) * CSLAB],
                                  W4a_d.ap()[:, ct * CSLAB:(ct + 1) * CSLAB])
                w4_loaded[ct] = True

            WC[1] = _load_wblock(Wtc_d, "wc_ts")
            load_slab(0)
            WC[0] = _load_wblock(Whc_d, "wc_hs")
            issue_wx()
            load_slab(1)
            issue_wx()
            issue_wx()

            def w4_view(m0, nm, kc, ct):
                # [128, m(nm), 128] slice of w4all at (kc, ct)
                return bass.AP(
                    tensor=w4all.tensor,
                    offset=w4all.offset + ct * CSLAB + m0 * KC * 128 + kc * 128,
                    ap=[w4all.ap[0], [KC * 128, nm], [1, 128]])

            # ---- phase E: entity pooling  ent = S^T @ seq ----
            ps_e0 = psg.tile([NENT, 512], F32, name="gen")
            ps_e1 = psg.tile([NENT, 256], F32, name="gen")
            for kc in range(LC):
                nc.tensor.matmul(ps_e0[:], S_t[:, kc, :], seq_t[:, kc, 0:512],
                                 start=(kc == 0), stop=(kc == LC - 1))
                nc.tensor.matmul(ps_e1[:], S_t[:, kc, :], seq_t[:, kc, 512:768],
                                 start=(kc == 0), stop=(kc == LC - 1))
            ent_nat = persist.tile([NENT, H], F32, name="ent_nat")
            nc.vector.tensor_scalar_mul(ent_nat[:, 0:512], ps_e0[:], 1.0)
            nc.vector.tensor_scalar_mul(ent_nat[:, 512:768], ps_e1[:], 1.0)

            entT = persist.tile([128, KC, NENT], F16, name="entT")
            for kc in range(KC):
                ps_tr = psg.tile([128, NENT], F32, name="gen")
                nc.tensor.transpose(ps_tr[:], ent_nat[:, kc * 128:(kc + 1) * 128],
                                    ident[:NENT, :NENT])
                nc.vector.tensor_scalar_mul(entT[:, kc, :], ps_tr[:], 1.0)

            # ---- phase A: A/B/C projections (batched over m) ----
            ABCD = []
            for ct in range(CT):
                ABCD.append(persist.tile([128, 4, NE2], F32, name=f"abcd{ct}"))

            ps_feat = [[psf.tile([128, 512], F32, name=f"pf{pt}a"),
                        psf.tile([128, 256], F32, name=f"pf{pt}b")]
                       for pt in range(PT)]

            XN = {}

            def emit_proj_mm(ct, half):
                # ps[41, 2, 128] = entT' @ [At|Bt] (half=1) or [Ah|Bh] ct-chunk
                m0 = 2 if half else 0
                ps_n = psg.tile([NENT, 256], F32, name="gen")
                for kc in range(KC):
                    nc.tensor.matmul(ps_n[:], entT[:, kc, :], w4_view(m0, 2, kc, ct),
                                     start=(kc == 0), stop=(kc == KC - 1))
                x_n = tmpp.tile([NENT, 256], F32, name=f"x_n{half}", bufs=2)
                nc.vector.tensor_scalar_mul(x_n[:], ps_n[:], 1.0)
                XN[(ct, half)] = x_n

            def emit_proj_tr(ct, half):
                x_n = XN[(ct, half)]
                for mi in range(2):
                    m = (2 if half else 0) + mi
                    ps_tr = psg.tile([128, NENT], F32, name="gen")
                    nc.tensor.transpose(ps_tr[:], x_n[:, mi * 128:(mi + 1) * 128],
                                        ident[:NENT, :NENT])
                    nc.vector.tensor_scalar_mul(ABCD[ct][:, m, 0:NENT], ps_tr[:], 1.0)

            CB = {}

            def emit_c_chain(side, bias_t):
                # C = cls @ WC[side]; broadcast [1,768] -> [128, CT] via PE
                # transposes (no DRAM round-trip).
                ps_c0 = psg.tile([NENT, 512], F32, name="gen")
                ps_c1 = psg.tile([NENT, 256], F32, name="gen")
                w_t = WC[side]
                for kc in range(KC):
                    nc.tensor.matmul(ps_c0[:1, :], entT[:, kc, IB + E:IB + E + 1],
                                     w_t[:, kc, 0:512],
                                     start=(kc == 0), stop=(kc == KC - 1))
                    nc.tensor.matmul(ps_c1[:1, :], entT[:, kc, IB + E:IB + E + 1],
                                     w_t[:, kc, 512:768],
                                     start=(kc == 0), stop=(kc == KC - 1))
                c_nat = cnp.tile([1, EMB], F32, name="c_nat")
                nc.vector.tensor_scalar_mul(c_nat[:, 0:512], ps_c0[:1, :], 1.0)
                nc.vector.tensor_scalar_mul(c_nat[:, 512:768], ps_c1[:1, :], 1.0)
                cb = persist.tile([128, CT], F32, name=f"cb{side}")
                for ct in range(CT):
                    ps_ctr = psg.tile([128, 1], F32, name="gen")
                    nc.tensor.transpose(ps_ctr[:], c_nat[:, ct * 128:(ct + 1) * 128],
                                        ident[:1, :1])
                    nc.vector.tensor_tensor(cb[:, ct:ct + 1], ps_ctr[:],
                                            bias_t[:, ct:ct + 1],
                                            op=mybir.AluOpType.add)
                CB[side] = cb

            def colview(tile_, m, col0, ap_pat):
                return bass.AP(tensor=tile_.tensor,
                               offset=tile_.offset + m * NE2 + col0,
                               ap=[tile_.ap[0]] + ap_pat)

            ts_dram = dramp.tile([EMB, 256], F16, name="ts_dram")
            hs_dram = dramp.tile([EMB, 256], F16, name="hs_dram")

            def emit_tanh(ct, ma, mb, cola, colb, side, dst_dram, dup_order):
                tmp = tmpp.tile([128, 8, 32], F32, name="tmp")
                nc.vector.tensor_tensor(
                    tmp[:], colview(ABCD[ct], ma, cola[0], cola[1]),
                    colview(ABCD[ct], mb, colb[0], colb[1]),
                    op=mybir.AluOpType.add)
                xt = hstp.tile([128, 256], F16, name="xt")
                nc.scalar.activation(
                    xt[:].rearrange("p (a b) -> p a b", a=8),
                    tmp[:], mybir.ActivationFunctionType.Tanh,
                    bias=CB[side][:, ct:ct + 1], scale=1.0)
                if dup_order:
                    for ph in range(2):
                        dst = bass.AP(
                            tensor=dst_dram.tensor,
                            offset=dst_dram.offset + (ct * 128 + ph * 64) * 256,
                            ap=[[256, 8], [8 * 256, 8], [1, 256]])
                        nc.scalar.dma_start(dst, xt[ph * 64:(ph + 1) * 64, :])
                else:
                    nc.scalar.dma_start(dst_dram[ct * 128:(ct + 1) * 128, :], xt[:])

            # tail constants, emitted mid-stream on the scalar queue.
            wgc_t = persist.tile([128, CT, NL], F16, name="wgc_t")
            qv_b = persist.tile([128, NL], F32, name="qv_b")
            rv_b = persist.tile([128, NL], F32, name="rv_b")
            bx_row = persist.tile([1, EMB], F16, name="bx_row")
            ones_r = persist.tile([1, 128], F16, name="ones_r")
            ident16 = persist.tile([128, 128], F16, name="ident16")

            def emit_tail_consts():
                nc.scalar.dma_start(
                    wgc_t[:], Wgc_d.ap().rearrange("(c p) n -> p c n", p=128))
                nc.scalar.dma_start(qv_b[:], qv_d.ap())
                nc.scalar.dma_start(rv_b[:], rv_d.ap())
                nc.scalar.dma_start(bx_row[:], bxr_d.ap())
                nc.vector.memset(ones_r[:], 1.0)
                nc.scalar.copy(ident16[:], ident[:])

            def emit_tanh_ts(ct):
                emit_tanh(ct, 2, 3, (IB, [[0, 8], [1, 32]]), (0, [[1, 8], [0, 32]]),
                          1, ts_dram, dup_order=False)

            def emit_tanh_hs(ct):
                emit_tanh(ct, 0, 1, (0, [[1, 8], [0, 32]]), (IB, [[0, 8], [1, 32]]),
                          0, hs_dram, dup_order=True)

            # per-ct chain pieces: ts side fully first, hs side after
            def emit_ct_piece(ct, s):
                if s == 0:
                    emit_proj_mm(ct, 1)
                elif s == 1:
                    emit_proj_tr(ct, 1)
                elif s == 2:
                    emit_tanh_ts(ct)
                elif s == 3:
                    emit_proj_mm(ct, 0)
                elif s == 4:
                    emit_proj_tr(ct, 0)
                elif s == 5:
                    emit_tanh_hs(ct)

            # head: ts-side C chain + ts pieces first, hs side after
            emit_c_chain(1, bt_t)
            for s in (0, 1, 2):
                emit_ct_piece(0, s)
            emit_c_chain(0, bh_t)
            for s in (3, 4, 5):
                emit_ct_piece(0, s)

            # ---- phase M: main contraction over W_ext ----
            def emit_hsdup_dma(hsdup, g, di):
                src = bass.AP(
                    tensor=hs_dram.tensor,
                    offset=hs_dram.offset + (g * 64 + di * 8) * 256,
                    ap=[[0, 16], [1, 8 * 256]])
                nc.scalar.dma_start(
                    hsdup[di * 16:(di + 1) * 16, :, :].rearrange(
                        "p l c -> p (l c)"), src)

            def emit_tsdup_load(tsdup, g):
                src = bass.AP(
                    tensor=ts_dram.tensor,
                    offset=ts_dram.offset + g * 64 * 256,
                    ap=[[256, 16], [16 * 256, 4], [1, 256]])
                nc.scalar.dma_start(tsdup[0:16, :, :], src)

            def emit_tsdup_double(tsdup, step):
                n = 16 << step
                nc.scalar.dma_start(tsdup[n:2 * n, :, :], tsdup[0:n, :, :])

            def alloc_group():
                return (hsdupp.tile([128, 8, 256], F16, name="hsdup"),
                        tsdp.tile([128, 4, 256], F16, name="tsdup"))

            cur = alloc_group()
            for di in range(8):
                emit_hsdup_dma(cur[0], 0, di)
            emit_tsdup_load(cur[1], 0)
            for st in range(3):
                emit_tsdup_double(cur[1], st)

            for g in range(G):
                nxt = alloc_group() if g + 1 < G else None
                ct_next = g // 2 + 1
                for ib2 in range(8):
                    if nxt is not None:
                        # spread the factor staging DMAs across the group
                        if ib2 < 4:
                            emit_hsdup_dma(nxt[0], g + 1, 2 * ib2)
                            emit_hsdup_dma(nxt[0], g + 1, 2 * ib2 + 1)
                            if ib2 == 0:
                                emit_tsdup_load(nxt[1], g + 1)
                        elif ib2 < 7:
                            emit_tsdup_double(nxt[1], ib2 - 4)
                    if g % 2 == 0 and ct_next < CT and 1 <= ib2 < 7:
                        emit_ct_piece(ct_next, ib2 - 1)
                    # stream the remaining projection slabs 2 groups ahead
                    if (g, ib2) in ((0, 7), (1, 3), (1, 7), (2, 3)):
                        nxt_slab = w4_loaded.index(False) if False in w4_loaded else None
                        if nxt_slab is not None:
                            load_slab(nxt_slab)
                    if g == 1 and ib2 == 6:
                        emit_tail_consts()

                    kt0 = g * 32 + ib2 * 4
                    bl4 = blp.tile([128, 4, 256], F16, name="bl4")
                    hs_b = bass.AP(tensor=cur[0].tensor,
                                   offset=cur[0].offset + ib2 * 256,
                                   ap=[cur[0].ap[0], [0, 4], [1, 256]])
                    nc.vector.tensor_tensor(bl4[:], hs_b, cur[1][:],
                                            op=mybir.AluOpType.mult)
                    # paced refill: at most 2 issues per iteration, ramping
                    # the ring up without a head-of-stream burst.
                    cidx = kt0 // WCH
                    target = min(cidx + WBUFS, NCHUNK)
                    n_iss = min(2, max(0, target - wx_next[0]))
                    for _ in range(n_iss):
                        issue_wx()
                    wx_ch = wx_tiles.pop(cidx)
                    for jb in range(4):
                        kt = kt0 + jb
                        kl = kt % WCH
                        for pt in range(PT):
                            lhsT = bl4[:, jb, pt * 128:(pt + 1) * 128]
                            nc.tensor.matmul(
                                ps_feat[pt][0][:], lhsT,
                                wx_ch[:, kl * EMB:kl * EMB + 512],
                                start=(kt == 0), stop=False)
                            nc.tensor.matmul(
                                ps_feat[pt][1][:], lhsT,
                                wx_ch[:, kl * EMB + 512:(kl + 1) * EMB],
                                start=(kt == 0), stop=False)
                cur = nxt

            # ---- phase L: bias (rank-1 matmul), relu, folded-LN classifier ----
            ln16 = []
            for pt in range(PT):
                nc.tensor.matmul(ps_feat[pt][0][:], ones_r[:], bx_row[:, 0:512],
                                 start=False, stop=True, skip_group_check=True)
                nc.tensor.matmul(ps_feat[pt][1][:], ones_r[:], bx_row[:, 512:768],
                                 start=False, stop=True, skip_group_check=True)
            for pt in range(PT):
                ln = persist.tile([128, EMB], F16, name=f"ln{pt}")
                nc.scalar.activation(ln[:, 0:512], ps_feat[pt][0][:],
                                     mybir.ActivationFunctionType.Relu,
                                     bias=0.0, scale=1.0)
                nc.scalar.activation(ln[:, 512:768], ps_feat[pt][1][:],
                                     mybir.ActivationFunctionType.Relu,
                                     bias=0.0, scale=1.0)
                ln16.append(ln)

            # DVE-only stats first so the ACT queue stays free for the
            # transpose copies; sqrt lands after each pt's copies.
            mvs = []
            for pt in range(PT):
                stats = tmpp.tile([128, 3, 6], F32, name="stats")
                f_re = ln16[pt].rearrange("p (c f) -> p c f", c=3)
                for c in range(3):
                    nc.vector.bn_stats(stats[:, c, :], f_re[:, c, :])
                mv = tmpp.tile([128, 2], F32, name="mv")
                nc.vector.bn_aggr(mv[:], stats[:])
                mvs.append(mv)

            for pt in range(PT):
                lnT = persist.tile([128, CT, 128], F16, name=f"lnT{pt}")
                for ct in range(CT):
                    ps_tr2 = psg.tile([128, 128], F16, name="gen")
                    nc.tensor.transpose(ps_tr2[:], ln16[pt][:, ct * 128:(ct + 1) * 128],
                                        ident16[:])
                    nc.scalar.copy(lnT[:, ct, :], ps_tr2[:])

                ps_lg = psg.tile([128, NL], F32, name="gen")
                for ct in range(CT):
                    nc.tensor.matmul(ps_lg[:], lnT[:, ct, :], wgc_t[:, ct, :],
                                     start=(ct == 0), stop=(ct == CT - 1))
                sd = tmpp.tile([128, 1], F32, name="sd")
                nc.scalar.activation(sd[:], mvs[pt][:, 1:2],
                                     mybir.ActivationFunctionType.Sqrt,
                                     bias=eps_t[:], scale=1.0)
                rstd = tmpp.tile([128, 1], F32, name="rstd")
                nc.vector.reciprocal(rstd[:], sd[:])
                mrs = tmpp.tile([128, 1], F32, name="mrs")
                nc.vector.tensor_tensor(mrs[:], mvs[pt][:, 0:1], rstd[:],
                                        op=mybir.AluOpType.mult)
                # logits = rstd*S1 - mrs*qv + rv
                t1 = tmpp.tile([128, NL], F32, name="t1")
                nc.vector.tensor_scalar(t1[:], ps_lg[:], rstd[:], None,
                                        op0=mybir.AluOpType.mult)
                t2 = tmpp.tile([128, NL], F32, name="t2")
                nc.vector.tensor_scalar(t2[:], qv_b[:], mrs[:], None,
                                        op0=mybir.AluOpType.mult)
                t3 = tmpp.tile([128, NL], F32, name="t3")
                nc.vector.tensor_tensor(t3[:], t1[:], t2[:],
                                        op=mybir.AluOpType.subtract)
                out_sb = tmpp.tile([128, NL], F32, name="out_sb")
                nc.vector.tensor_tensor(out_sb[:], t3[:], rv_b[:],
                                        op=mybir.AluOpType.add)
                nc.scalar.dma_start(out_d.ap()[pt * 128:(pt + 1) * 128, :], out_sb[:])

    nc.compile()
    return nc


_NC_CACHE = []


def _get_module():
    if not _NC_CACHE:
        _NC_CACHE.append(_build_module())
    return _NC_CACHE[0]


def _build_inputs(seq, starts, ends, mention_mask, W_head, b_head, W_tail, b_tail,
                  W_ext, b_ext, ln_g, ln_b, W_cls):
    seq = np.asarray(seq, np.float32)
    starts = np.asarray(starts, np.int64)
    ends = np.asarray(ends, np.int64)
    mask = np.asarray(mention_mask, np.float32)

    S_b = np.zeros((B, L, E), np.float32)
    denom = np.maximum(mask.sum(axis=2), 1.0)          # [B, E]
    w = mask * 0.5 / denom[:, :, None]                 # [B, E, M]
    for b in range(B):
        for e in range(E):
            np.add.at(S_b[b, :, e], starts[b, e] + 1, w[b, e])
            np.add.at(S_b[b, :, e], ends[b, e], w[b, e])

    cls_col = np.zeros((L, 1), np.float32)
    cls_col[0, 0] = 1.0

    ln_g32 = np.asarray(ln_g, np.float32)
    ln_b32 = np.asarray(ln_b, np.float32)
    Wc32 = np.asarray(W_cls, np.float32)
    wgc = (ln_g32[:, None] * Wc32).astype(np.float16)
    qv = (ln_g32[None, :] @ Wc32).astype(np.float32)   # [1, NL]
    rv = (ln_b32[None, :] @ Wc32).astype(np.float32)

    Wh16 = np.asarray(W_head, np.float32).astype(np.float16)
    Wt16 = np.asarray(W_tail, np.float32).astype(np.float16)
    # W4a ct-major: [m, kc, 128p, ct, 128c] -> [128p, ct, m, kc, 128c]
    w4a = np.stack([Wh16[0:H], Wh16[H:2 * H], Wt16[0:H], Wt16[H:2 * H]])
    w4a = (w4a.reshape(4, KC, 128, CT, 128).transpose(2, 3, 0, 1, 4)
           .reshape(128, -1))

    shared = {
        "Whc": np.ascontiguousarray(Wh16[2 * H:3 * H]),
        "Wtc": np.ascontiguousarray(Wt16[2 * H:3 * H]),
        "W4a": np.ascontiguousarray(w4a),
        "bh": np.ascontiguousarray(np.asarray(b_head, np.float32).reshape(CT, 128).T),
        "bt": np.ascontiguousarray(np.asarray(b_tail, np.float32).reshape(CT, 128).T),
        "Wx": np.ascontiguousarray(
            np.asarray(W_ext).astype(np.float16)
            .reshape(G, 8, 8, 4, 16, EMB).transpose(2, 4, 0, 1, 3, 5)
            .reshape(128, KT * EMB)),
        "bxr": np.ascontiguousarray(np.asarray(b_ext, np.float32).astype(np.float16).reshape(1, EMB)),
        "Wgc": np.ascontiguousarray(wgc),
        "qv": np.ascontiguousarray(np.broadcast_to(qv, (128, NL))),
        "rv": np.ascontiguousarray(np.broadcast_to(rv, (128, NL))),
    }
    # compact seq to the rows the pooling matrix actually touches
    rows = [None] * B
    for b in range(B):
        r = np.unique(np.concatenate(
            [starts[b].ravel() + 1, ends[b].ravel(), np.array([0])]))
        assert len(r) <= LP
        rows[b] = r

    in_maps = []
    for core in range(N_CORES):
        b, ib = core // 4, core % 4
        r = rows[b]
        seq_c = np.zeros((LP, H), np.float16)
        seq_c[:len(r)] = seq[b][r].astype(np.float16)
        S_full = np.concatenate(
            [S_b[b][:, ib * IB:(ib + 1) * IB], S_b[b], cls_col], axis=1)
        S_c = np.zeros((LP, NENT), np.float16)
        S_c[:len(r)] = S_full[r].astype(np.float16)
        in_maps.append({
            "seq": np.ascontiguousarray(seq_c),
            "S": np.ascontiguousarray(S_c),
            **shared,
        })
    return in_maps


def kernel(**inputs) -> np.ndarray:
    nc = _get_module()
    in_maps = _build_inputs(**inputs)
    res = run_bass_kernel_spmd(nc, in_maps, core_ids=list(range(N_CORES)))
    outs = np.stack([res.results[c]["out"] for c in range(N_CORES)])  # [8,256,97]
    return outs.reshape(B, 4, IB, E, NL).reshape(B, E, E, NL)
